# revision 4
# baseline (speedup 1.0000x reference)
"""Distributed real SHT (spherical harmonic transform) on 8 trn2 NeuronCores.

  out[b,c,l,m] = sum_k W[m,l,k] * XF[b,c,m,k],  XF = (2*pi/nlon)*rfft(x, lon)[:mmax]

Two launches with a free host exchange between them.

Stage A (channel-sharded DFT): the longitude DFT is folded twice:
  fold n <-> 720-n      -> cos (RE) / sin (IM) halves, 361/359 rows
  fold n' <-> 360-n'    -> m-even / m-odd classes, ~181/180 rows each
so each psum tile takes 3 matmuls instead of 9: two 128-row mains (m-even,
m-odd col halves) and ONE merged block-diagonal tail (both classes' last
<=53 contraction rows packed at partition offsets 0/64).  Latitude k is
"pole-folded": pairs (k, 360-k) for k<128 become even/odd combinations
(P_l^m(-x) = (-1)^(l+m) P_l^m(x) makes stage B contract each l against one
parity only), the 105-row equator band stays raw.  Output tiles are stored
with 1.4KB DMA lines.

Stage B (m-sharded Legendre, m interleaved mod 8): per index i the
contraction is [e-pole rows flo..127 | o-pole rows | equator window], and
the l range [8i, 361) is split into parity classes packed [e: 0..H | o:
LeP..LeP+H] in psum (LeP = H rounded to 64 so partition offsets stay
legal).  Weights below the window are exactly zero and never loaded.
bf16 operands keep the PE at 2.4 GHz; psum accumulation is fp32.
"""

import os

import numpy as np

import concourse.bacc as bacc
import concourse.mybir as mybir
from concourse.tile import TileContext
from concourse.bass_utils import run_bass_kernel_spmd

LAST_PERF = {}

NLAT = 361
NLON = 720
MMAX = 361
LMAX = 361
C = 256
NCORES = 8
CPC = C // NCORES  # 32 channels per core
NIDX = 46          # m indices per core (m = 8i + j)
NME = 181          # even-m class size
NMO = 180          # odd-m class size
EQN = 105          # equator band rows (k = 128..232)
XCOLS = 6 * 361    # stage A input free cols per channel
MCOLS = 1444       # stage A DFT-matrix free cols

# per-index folded k-window start (min over the group's 8 m's of the first
# latitude where max_l |W[m,l,k]| is non-negligible, folded about k=180)
FLO = [0, 1, 2, 4, 6, 8, 10, 12, 14, 17, 19, 21, 24, 26, 28, 31, 33, 36,
       38, 41, 43, 46, 48, 51, 54, 57, 59, 62, 65, 68, 71, 75, 78, 81, 85,
       88, 92, 96, 100, 105, 109, 114, 120, 127, 134, 147]

F32 = mybir.dt.float32
BF16 = mybir.dt.bfloat16


def _bh(i):
    """Per-index geometry: class size H, pole rows pw, equator window."""
    H = 181 - 4 * i
    flo = FLO[i]
    pw = 128 - min(flo, 128)
    elo = max(0, flo - 128)
    ew = 105 - 2 * elo
    return H, pw, elo, ew


def b_order(n):
    """Interleave heavy (small i) and light iterations; lightest last."""
    order = []
    lo, hi = 0, n - 2
    while lo <= hi:
        order.append(lo)
        if hi != lo:
            order.append(hi)
        lo += 1
        hi -= 1
    order.append(n - 1)
    return order


def build_stage_a():
    """xin [cpc,128,2166]: col-blocks of 361 k-cols ([lat-e 128|lat-o 128|
    raw 105]): B0 ce-main, B1 co-main, B2 se-main, B3 so-main, B4 cos-tails
    (rows 0:53 ce, 64:116 co), B5 sin-tails (0:51 se, 64:116 so).
    mats [128,1444]: M0 ce[0:181], M1 co[181:361], M2 se[361:542],
    M3 so[542:722], M4 cos-tail blockdiag [722:1083], M5 sin [1083:1444].
    xfp [cpc,128,1444]: per ri 722 cols = [e-pole 361 m | o-pole 361 m].
    xfq [cpc,105,722]: equator band raw, [RE 361 | IM 361]."""
    nc = bacc.Bacc("TRN2", target_bir_lowering=False)
    xin = nc.dram_tensor("xin", [CPC, 128, XCOLS], BF16, kind="ExternalInput")
    mats = nc.dram_tensor("mats", [128, MCOLS], BF16, kind="ExternalInput")
    xfp = nc.dram_tensor("xfp", [CPC, 128, 1444], BF16, kind="ExternalOutput")
    xfq = nc.dram_tensor("xfq", [CPC, EQN, 722], BF16, kind="ExternalOutput")

    ktiles = [(0, 128), (128, 128), (256, 105)]
    with TileContext(nc) as tc:
        with (
            tc.tile_pool(name="mats", bufs=1) as matp,
            tc.tile_pool(name="xinp", bufs=6) as xinp,
            tc.tile_pool(name="outp", bufs=8) as outp,
            tc.tile_pool(name="ps", bufs=7, space="PSUM") as psp,
        ):
            mt = matp.tile([128, MCOLS], BF16, tag="mats")
            nc.scalar.dma_start(out=mt, in_=mats[:, :])
            cp_eng = [nc.vector, nc.scalar]
            st_eng = [nc.gpsimd, nc.scalar, nc.sync]
            ci = si = 0
            for c in range(CPC):
                xt = xinp.tile([128, XCOLS], BF16, tag="xin")
                nc.sync.dma_start(out=xt, in_=xin[c])
                qo = outp.tile([128, 722], BF16, tag="qo")
                for ri in range(2):
                    po = outp.tile([128, 722], BF16, tag="po")
                    be = (2 * ri) * 361
                    bo = (2 * ri + 1) * 361
                    bt = (4 + ri) * 361
                    me0 = (0, 361)[ri]
                    mo0 = (181, 542)[ri]
                    mt0 = (722, 1083)[ri]
                    for t, (k0, kw) in enumerate(ktiles):
                        ps = psp.tile([128, 361], F32, tag="ps")
                        nc.tensor.matmul(
                            ps[:kw, 0:181],
                            xt[:, be + k0 : be + k0 + kw],
                            mt[:, me0 : me0 + 181],
                            start=True, stop=False,
                        )
                        # start=False: the m-even main's start already marked
                        # the whole 2KB psum bank pending-zero (start zeroes
                        # the full zero-region, not just the addressed cols)
                        nc.tensor.matmul(
                            ps[:kw, 181:361],
                            xt[:, bo + k0 : bo + k0 + kw],
                            mt[:, mo0 : mo0 + 180],
                            start=False, stop=False,
                        )
                        nc.tensor.matmul(
                            ps[:kw, 0:361],
                            xt[0:116, bt + k0 : bt + k0 + kw],
                            mt[0:116, mt0 : mt0 + 361],
                            start=False, stop=True,
                        )
                        if t < 2:
                            dst = po[:kw, t * 361 : (t + 1) * 361]
                        else:
                            dst = qo[:kw, ri * 361 : ri * 361 + 361]
                        eng = cp_eng[ci % 2]
                        if eng is nc.vector:
                            eng.tensor_copy(out=dst, in_=ps[:kw, 0:361])
                        else:
                            eng.copy(dst, ps[:kw, 0:361])
                        ci += 1
                    st_eng[si % 3].dma_start(
                        out=xfp[c][:, ri * 722 : (ri + 1) * 722], in_=po
                    )
                    si += 1
                st_eng[si % 3].dma_start(out=xfq[c], in_=qo[:EQN, :])
                si += 1
    nc.compile()
    return nc


def build_stage_b():
    """xfb [46,361,512]: rows [0:pw) e-pole, [pw:2pw) o-pole, [2pw:2pw+ew)
    equator window; cols [RE 256ch | IM 256ch].  wt [46,361,362]: same rows;
    pole cols [0:H) = class-l weights, equator cols [0:2H) = [e | o].
    out [46,362,512]: rows [0:H) e-class l's, [H:2H) o-class."""
    nc = bacc.Bacc("TRN2", target_bir_lowering=False)
    xfb = nc.dram_tensor("xfb", [NIDX, 361, 512], BF16, kind="ExternalInput")
    wt = nc.dram_tensor("wt", [NIDX, 361, 362], BF16, kind="ExternalInput")
    out = nc.dram_tensor("out", [NIDX, 362, 512], BF16, kind="ExternalOutput")

    order = b_order(NIDX)
    with TileContext(nc) as tc:
        with (
            tc.tile_pool(name="rhs", bufs=6) as rhsp,
            tc.tile_pool(name="wts", bufs=6) as wtp,
            tc.tile_pool(name="outp", bufs=8) as outp,
            tc.tile_pool(name="ps", bufs=7, space="PSUM") as psp,
        ):
            st_eng = [nc.gpsimd, nc.sync, nc.scalar]
            si = 0
            for bi in range(NIDX):
                i = order[bi]
                H, pw, elo, ew = _bh(i)
                rt = rhsp.tile([128, 3 * 512], BF16, tag="rhs")
                if pw:
                    nc.sync.dma_start(out=rt[:pw, 0:512], in_=xfb[i, 0:pw, :])
                    nc.sync.dma_start(
                        out=rt[:pw, 512:1024], in_=xfb[i, pw : 2 * pw, :]
                    )
                nc.sync.dma_start(
                    out=rt[:ew, 1024 : 1024 + 512],
                    in_=xfb[i, 2 * pw : 2 * pw + ew, :],
                )
                wtile = wtp.tile([128, 4 * H], BF16, tag="wt")
                if pw:
                    nc.scalar.dma_start(out=wtile[:pw, 0:H], in_=wt[i, 0:pw, 0:H])
                    nc.scalar.dma_start(
                        out=wtile[:pw, H : 2 * H], in_=wt[i, pw : 2 * pw, 0:H]
                    )
                nc.scalar.dma_start(
                    out=wtile[:ew, 2 * H : 4 * H],
                    in_=wt[i, 2 * pw : 2 * pw + ew, 0 : 2 * H],
                )
                LeP = -(-H // 64) * 64
                Lt = LeP + H
                for a in range(0, Lt, 128):
                    b = min(a + 128, Lt)
                    ps = psp.tile([128, 512], F32, tag="ps")
                    ot = outp.tile([128, 512], BF16, tag="ot")
                    el = max(0, min(b, H) - a)
                    if el:
                        if pw:
                            nc.tensor.matmul(
                                ps[0:el, :], wtile[:pw, a : a + el],
                                rt[:pw, 0:512], start=True, stop=False,
                            )
                        nc.tensor.matmul(
                            ps[0:el, :],
                            wtile[:ew, 2 * H + a : 2 * H + a + el],
                            rt[:ew, 1024 : 1024 + 512],
                            start=(pw == 0), stop=True,
                        )
                    o0 = max(LeP, a)
                    if b > o0:
                        ol = o0 - a
                        lo0 = o0 - LeP
                        olen = b - o0
                        if pw:
                            nc.tensor.matmul(
                                ps[ol : ol + olen, :],
                                wtile[:pw, H + lo0 : H + lo0 + olen],
                                rt[:pw, 512:1024], start=True, stop=False,
                            )
                        nc.tensor.matmul(
                            ps[ol : ol + olen, :],
                            wtile[:ew, 3 * H + lo0 : 3 * H + lo0 + olen],
                            rt[:ew, 1024 : 1024 + 512],
                            start=(pw == 0), stop=True,
                        )
                    if el:
                        nc.vector.tensor_copy(out=ot[0:el, :], in_=ps[0:el, :])
                        st_eng[si % 3].dma_start(
                            out=out[i, a : a + el, :], in_=ot[0:el, :]
                        )
                        si += 1
                    if b > o0:
                        nc.vector.tensor_copy(
                            out=ot[ol : ol + olen, :], in_=ps[ol : ol + olen, :]
                        )
                        st_eng[si % 3].dma_start(
                            out=out[i, H + lo0 : H + lo0 + olen, :],
                            in_=ot[ol : ol + olen, :],
                        )
                        si += 1
    nc.compile()
    return nc


def _dft_slabs():
    """Folded DFT matrices (f64 -> bf16 via f32).
    Returns Mce [181,181], Mco [180,180], Mse [179,181], Mso [180,180]."""
    s = 2.0 * np.pi / NLON
    me = np.arange(0, MMAX, 2)   # 181 even m
    mo = np.arange(1, MMAX, 2)   # 180 odd m

    def cosm(rows, ms):
        ang = 2.0 * np.pi * ((np.outer(rows, ms)) % NLON) / NLON
        return s * np.cos(ang)

    def sinm(rows, ms):
        ang = 2.0 * np.pi * ((np.outer(rows, ms)) % NLON) / NLON
        return -s * np.sin(ang)

    Mce = cosm(np.arange(181), me)
    Mco = cosm(np.arange(180), mo)
    Mse = sinm(np.arange(1, 180), me)
    Mso = sinm(np.r_[np.arange(1, 180), 180], mo)
    return Mce, Mco, Mse, Mso


def _fold_lon(x):
    """x (C, 361, 720) -> ce (C,361,181), co (C,361,180), se (C,361,179),
    so (C,361,180): double lon folding, f32."""
    xc = np.empty((x.shape[0], NLAT, 361), dtype=np.float32)
    xc[..., 0] = x[..., 0]
    xc[..., 1:360] = x[..., 1:360] + x[..., :360:-1]
    xc[..., 360] = x[..., 360]
    xs = x[..., 1:360] - x[..., :360:-1]  # n' = 1..359
    ce = np.empty((x.shape[0], NLAT, 181), dtype=np.float32)
    ce[..., 0:180] = xc[..., 0:180] + xc[..., 360:180:-1]
    ce[..., 180] = xc[..., 180]
    co = xc[..., 0:180] - xc[..., 360:180:-1]
    # xs index j maps to n' = j+1 (rows 0..358)
    se = xs[..., 0:179] - xs[..., 358:179:-1]          # n' = 1..179
    so = np.concatenate(
        [xs[..., 0:179] + xs[..., 358:179:-1], xs[..., 179:180]], axis=-1
    )  # n' = 1..179 then center n'=180
    return ce, np.ascontiguousarray(co), se, so


def _fold_lat(slab):
    """slab (C, 361, R) -> (C, 361, R) with k-cols [e 128 | o 128 | raw 105]
    ... wait: fold acts on axis 1 (k). Returns (C, 361, R) same rows."""
    e = slab[:, 0:128, :] + slab[:, 360:232:-1, :]
    o = slab[:, 0:128, :] - slab[:, 360:232:-1, :]
    raw = slab[:, 128:233, :]
    return np.concatenate([e, o, raw], axis=1)  # (C, 361, R)


def pack_stage_a(x):
    """x (C,361,720) f32 -> xin (C,128,2166) bf16, mats (128,1444) bf16."""
    import ml_dtypes

    bf = ml_dtypes.bfloat16
    ce, co, se, so = _fold_lon(x)
    # lat-fold each slab along k, then transpose to [rows(contraction), k]
    slabs = []
    for s in (ce, co, se, so):
        f = _fold_lat(s)                    # (C, 361k, R)
        slabs.append(f.transpose(0, 2, 1))  # (C, R, 361k)
    xin = np.zeros((x.shape[0], 128, XCOLS), dtype=bf)
    # mains
    for idx, s in enumerate(slabs):
        xin[:, :, idx * 361 : (idx + 1) * 361] = s[:, 0:128, :].astype(bf)
    # tails: B4 cos (ce rows 128:181 at p0, co 128:180 at p64)
    xin[:, 0:53, 4 * 361 : 5 * 361] = slabs[0][:, 128:181, :].astype(bf)
    xin[:, 64:116, 4 * 361 : 5 * 361] = slabs[1][:, 128:180, :].astype(bf)
    xin[:, 0:51, 5 * 361 : 6 * 361] = slabs[2][:, 128:179, :].astype(bf)
    xin[:, 64:116, 5 * 361 : 6 * 361] = slabs[3][:, 128:180, :].astype(bf)

    Mce, Mco, Mse, Mso = _dft_slabs()
    mats = np.zeros((128, MCOLS), dtype=bf)
    mats[:, 0:181] = Mce[0:128].astype(bf)
    mats[:, 181:361] = Mco[0:128].astype(bf)
    mats[:, 361:542] = Mse[0:128].astype(bf)
    mats[:, 542:722] = Mso[0:128].astype(bf)
    mats[0:53, 722:903] = Mce[128:181].astype(bf)
    mats[64:116, 903:1083] = Mco[128:180].astype(bf)
    mats[0:51, 1083:1264] = Mse[128:179].astype(bf)
    mats[64:116, 1264:1444] = Mso[128:180].astype(bf)
    return xin, mats


def _class_ls(i, j):
    """(e_class, o_class) l lists for index i on core j, padded to H."""
    H = 181 - 4 * i
    m = 8 * i + j
    lref = 8 * i
    ls = np.arange(lref, LMAX)
    e = ls[(ls + m) % 2 == 0]
    o = ls[(ls + m) % 2 == 1]
    return e, o, H


def pack_stage_b(weights, xfr_e, xfr_o, xfr_q, xfi_e, xfi_o, xfi_q):
    """weights (m,l,k) f32; xf*_e/[o] (C,128,361m) f32-ish arrays (pole
    folded values, m in class order [even|odd]); xf*_q (C,105,361m).
    Returns per-core in_maps list."""
    import ml_dtypes

    bf = ml_dtypes.bfloat16
    morder = np.r_[np.arange(0, MMAX, 2), np.arange(1, MMAX, 2)]
    minv = np.empty(MMAX, dtype=np.int64)
    minv[morder] = np.arange(MMAX)

    in_maps = []
    for j in range(NCORES):
        xfb = np.zeros((NIDX, 361, 512), dtype=bf)
        wtj = np.zeros((NIDX, 361, 362), dtype=bf)
        for i in range(NIDX):
            m = 8 * i + j
            if m >= MMAX:
                continue
            H, pw, elo, ew = _bh(i)
            mc = minv[m]
            plo = 128 - pw
            if pw:
                xfb[i, 0:pw, 0:256] = xfr_e[:, plo:128, mc].T
                xfb[i, 0:pw, 256:512] = xfi_e[:, plo:128, mc].T
                xfb[i, pw : 2 * pw, 0:256] = xfr_o[:, plo:128, mc].T
                xfb[i, pw : 2 * pw, 256:512] = xfi_o[:, plo:128, mc].T
            xfb[i, 2 * pw : 2 * pw + ew, 0:256] = xfr_q[:, elo : elo + ew, mc].T
            xfb[i, 2 * pw : 2 * pw + ew, 256:512] = xfi_q[:, elo : elo + ew, mc].T
            e_ls, o_ls, _ = _class_ls(i, j)
            We = weights[m][e_ls]  # (ne, 361k)
            Wo = weights[m][o_ls]
            if pw:
                wtj[i, 0:pw, 0 : len(e_ls)] = We[:, plo:128].T.astype(bf)
                wtj[i, pw : 2 * pw, 0 : len(o_ls)] = Wo[:, plo:128].T.astype(bf)
            eq = slice(128 + elo, 128 + elo + ew)
            wtj[i, 2 * pw : 2 * pw + ew, 0 : len(e_ls)] = We[:, eq].T.astype(bf)
            wtj[i, 2 * pw : 2 * pw + ew, H : H + len(o_ls)] = Wo[:, eq].T.astype(bf)
        in_maps.append({"xfb": xfb, "wt": wtj})
    return in_maps


def _install_ntff_hook():
    import sys

    if "antenv.axon_hooks" in sys.modules:
        return
    import types

    mod = types.ModuleType("antenv.axon_hooks")
    state = {"hook": None}
    mod.set_axon_ntff_profile_hook = lambda h: state.__setitem__("hook", h)
    mod.get_axon_ntff_profile_hook = lambda: state["hook"]
    sys.modules["antenv.axon_hooks"] = mod
    try:
        import importlib.util as ilu

        spec = ilu.spec_from_file_location(
            "_trn_boot_hook", "/root/.axon_site/trn_agent_boot/trn_boot.py"
        )
        tb = ilu.module_from_spec(spec)
        spec.loader.exec_module(tb)
        mod.set_axon_ntff_profile_hook(
            tb._ntff_profile_via_ctypes("/opt/axon/libaxon_pjrt.so")
        )
    except Exception:
        pass


def _run(nc, in_maps, label):
    kw = {}
    if os.environ.get("SHT_TRACE"):
        import concourse.bass_utils as bu

        bu.upload_artifacts = lambda tmpdir: tmpdir
        _install_ntff_hook()
        kw = dict(trace=True)
    try:
        res = run_bass_kernel_spmd(nc, in_maps, core_ids=list(range(NCORES)), **kw)
    except Exception:
        if not kw:
            raise
        res = run_bass_kernel_spmd(nc, in_maps, core_ids=list(range(NCORES)))
    LAST_PERF[label] = res.exec_time_ns
    return res


def kernel(x, weights):
    x = np.asarray(x, dtype=np.float32).reshape(C, NLAT, NLON)
    weights = np.asarray(weights, dtype=np.float32)

    xin, mats = pack_stage_a(x)
    nc_a = build_stage_a()
    in_maps = [
        {"xin": xin[j * CPC : (j + 1) * CPC], "mats": mats} for j in range(NCORES)
    ]
    res_a = _run(nc_a, in_maps, "stage_a")

    # gather stage A outputs -> full-channel arrays (class-ordered m cols)
    xfp = np.concatenate(
        [np.asarray(r["xfp"]) for r in res_a.results], axis=0
    ).astype(np.float32)  # (C, 128, 1444)
    xfq = np.concatenate(
        [np.asarray(r["xfq"]) for r in res_a.results], axis=0
    ).astype(np.float32)  # (C, 105, 722)
    xfr_e = xfp[:, :, 0:361]
    xfr_o = xfp[:, :, 361:722]
    xfi_e = xfp[:, :, 722:1083]
    xfi_o = xfp[:, :, 1083:1444]
    xfr_q = xfq[:, :, 0:361]
    xfi_q = xfq[:, :, 361:722]

    in_maps_b = pack_stage_b(weights, xfr_e, xfr_o, xfr_q, xfi_e, xfi_o, xfi_q)
    nc_b = build_stage_b()
    res_b = _run(nc_b, in_maps_b, "stage_b")

    out = np.zeros((1, C, LMAX, MMAX), dtype=np.complex64)
    for j in range(NCORES):
        o = np.asarray(res_b.results[j]["out"], dtype=np.float32)  # (46,362,512)
        for i in range(NIDX):
            m = 8 * i + j
            if m >= MMAX:
                continue
            H, pw, elo, ew = _bh(i)
            e_ls, o_ls, _ = _class_ls(i, j)
            oe = o[i, 0 : len(e_ls)]
            oo = o[i, H : H + len(o_ls)]
            out[0, :, e_ls, m] = (oe[:, 0:256] + 1j * oe[:, 256:512])
            out[0, :, o_ls, m] = (oo[:, 0:256] + 1j * oo[:, 256:512])
    return out
